# revision 20
# baseline (speedup 1.0000x reference)
"""Trainium2 Bass kernel for nn_DSC_86071144612259.

The reference network collapses to a single linear contraction

    u[b, c] = sum_{d<128} sum_{p} W[d, p, c] * y_rev[b, d, p]

where W [128, P, MC] is assembled exactly (float64, on host) from the
small parameter tensors.  The 270 MB y_rev stream is the real work and
is purely DMA bound, so the kernel moves y as *int8* (per-batch-row
scale, absmax/127) -- half the HBM traffic of the fp16 baseline.  The
PE only eats float dtypes (the BIR verifier rejects integer matmuls),
so int8 y is upconverted to fp16 on-chip: the sync HWDGE ring streams
int8 (measured ~375 GB/s with 2 MB descriptors, the per-core HBM
share), and the casts are split between DVE tensor_copy (~1.22
us/chunk) and ACT activation-Copy (~2.0 us/chunk), which together
(~1.32 chunks/us) hide under the stream (~0.7 us/chunk).

The tensor engine chases per chunk with fp16 matmuls accumulating in
fp32 PSUM (4 batch blocks concurrently in disjoint 32-column PE
groups); the per-row dequant scale is applied by the final PSUM->SBUF
tensor_mul, fused with the output copy.  The only numeric loss is the
int8 rounding of y (measured absmax-rel ~9.2e-3 < the 2e-2 gate).

Sharding: pure data parallel over the batch axis across 8 cores (2048
rows each); W and the scale tile are replicated per-core inputs.
"""

import numpy as np

B = 16384      # batch
L = 129        # history length of y_rev
P = 32         # observation dim
MC = 16        # control dim (output)
H = 24         # spectral dim
M = 64         # filter length
NCORES = 8
BS = B // NCORES           # 2048 batch rows per core
KD = 128                   # delays with nonzero weight
K = KD * P                 # 4096 contraction length
NKC = K // 128             # 32 k-chunks of 128 partitions
CW = BS                    # SBUF columns per chunk (2048)
NFREE = 512                # matmul moving free dim (one fp32 PSUM bank)
NB = BS // NFREE           # 4 batch chunks per core

# Chunks 0..NF16-1 travel as *fp16* (pre-divided by the row scale on
# host) on the ACT HWDGE ring (Q10) -- the two HWDGE rings sustain
# ~439 GB/s combined vs ~375 solo, so these bytes ride for free and
# skip conversion.  Chunks NF16..31 arrive int8 on the sync ring (Q1)
# and are cast to fp16 by DVE/ACT.  Q1 group sizes ramp 1->8->1: fine
# granularity at head (casts start immediately) and tail (short
# critical path), 2 MB descriptors in the bulk (the ring only reaches
# full rate with large transfers).
NF16 = 4
F16_GROUPS = [[0, 1], [2, 3]]
SYNC_GROUPS = [[4], [5, 6], [7, 8, 9, 10], [11, 12, 13, 14, 15, 16, 17, 18],
               [19, 20, 21, 22, 23, 24, 25, 26], [27, 28], [29, 30], [31]]

# Cast runs: per arrival group, one contiguous run per engine, one
# tensor_copy/activation op per run (batching amortizes the ~150-290 ns
# per-op overhead).  Measured rates: DVE ~1090 ns/chunk (2x mode), ACT
# ~1800 ns/chunk batched -> DVE 18 / ACT 10.  GpSimd casts are NOT
# used: ~8 us/chunk AND they drag concurrent DVE casts down to the
# same pace (measured).  GpSimd DMAs (SWDGE Q0) are also out: any Q0
# traffic collapses the concurrent HWDGE stream (measured 134+161 vs
# 375 solo).  The tail chunks go to DVE (faster) to shorten the tail.
CONV_RUNS = [
    ("vector", [4], 0),
    ("vector", [5, 6], 1),
    ("vector", [7, 8], 2), ("scalar", [9, 10], 2),
    ("vector", [11, 12, 13, 14], 3), ("scalar", [15, 16, 17, 18], 3),
    ("vector", [19, 20, 21, 22], 4), ("scalar", [23, 24, 25, 26], 4),
    ("vector", [27, 28], 5),
    ("vector", [29, 30], 6),
    ("vector", [31], 7),
]
CONV_ENGINES = ("vector", "scalar")
_CACHE = {}


def _build_w(M0, M_tilde, M_0l, M_big, sigma, lambda_e, phi, phi_tilde):
    """Collapse the parameter tensors into W [KD, MC, P] (float64).

    Mirrors reference.py exactly:
      term1: delay 0,      M0
      term2: delays 1..64, sum_i lambda_i^0.25 phi_tilde[j-1,i] M_tilde[i]
      term3: delays 0..63, sum_l sigma_l^0.25  phi[k,l]         M_0l[l]
      term4: delays 1..127 via conv(phi_tilde[:,i], phi[:,l]) and M_big
    """
    f8 = np.float64
    M0 = M0.astype(f8)
    M_tilde = M_tilde.astype(f8)
    M_0l = M_0l.astype(f8)
    M_big = M_big.astype(f8)
    sigma = sigma.astype(f8)
    lambda_e = lambda_e.astype(f8)
    phi = phi.astype(f8)
    phi_tilde = phi_tilde.astype(f8)

    W = np.zeros((KD, MC, P), dtype=f8)
    W[0] += M0
    pt = phi_tilde * (lambda_e ** 0.25)[None, :]
    W[1:M + 1] += np.einsum("ji,icp->jcp", pt, M_tilde)
    ps = phi * (sigma ** 0.25)[None, :]
    W[0:M] += np.einsum("kl,lcp->kcp", ps, M_0l)
    W4 = np.empty((H, H, 2 * M - 1), dtype=f8)
    for i in range(H):
        for l in range(H):
            W4[i, l] = np.convolve(phi_tilde[:, i], phi[:, l])
    scale = (lambda_e[:, None] * sigma[None, :]) ** 0.25
    W[1:2 * M] += np.einsum("ild,ilcp->dcp", W4 * scale[:, :, None], M_big)
    return W


def _get_nc():
    """Build the per-core Bass program (cached)."""
    if "nc" in _CACHE:
        return _CACHE["nc"]
    import concourse.bass as bass
    import concourse.mybir as mybir

    # per-chunk: (engine, run-ordinal on that engine) for matmul waits
    chunk_wait = {}
    runs_of = {e: [] for e in CONV_ENGINES}
    for ename, chunks, gi in CONV_RUNS:
        runs_of[ename].append((chunks, gi))
        for ci in chunks:
            chunk_wait[ci] = (ename, len(runs_of[ename]))
    assert sorted(chunk_wait) == list(range(NF16, NKC))

    nc = bass.Bass("TRN2", target_bir_lowering=False, enable_partition_id=False)
    y8 = nc.dram_tensor("y8", [128, (NKC - NF16) * CW], mybir.dt.int8,
                        kind="ExternalInput")
    yf = nc.dram_tensor("yf", [128, NF16 * CW], mybir.dt.float16,
                        kind="ExternalInput")
    w = nc.dram_tensor("w", [128, NKC * MC], mybir.dt.float16, kind="ExternalInput")
    s = nc.dram_tensor("s", [128, NFREE], mybir.dt.float32, kind="ExternalInput")
    ut = nc.dram_tensor("ut", [128, NFREE], mybir.dt.float16, kind="ExternalOutput")

    y8_sb = nc.alloc_sbuf_tensor("y8_sb", [128, (NKC - NF16) * CW], mybir.dt.int8)
    y_sb = nc.alloc_sbuf_tensor("y_sb", [128, NKC * CW], mybir.dt.float16)
    # W pre-swizzled on host: w_sb[p, ki*MC + c] = W_flat[ki*128 + p, c]
    w_sb = nc.alloc_sbuf_tensor("w_sb", [128, NKC * MC], mybir.dt.float16)
    # Dequant tile: s_sb[32*bc + c, j] = s_row[bc*512 + j]
    s_sb = nc.alloc_sbuf_tensor("s_sb", [128, NFREE], mybir.dt.float32)
    # Output striped across partitions: row 32*bc + c holds u^T[c, bc*512+j]
    u_sb = nc.alloc_sbuf_tensor("u_sb", [128, NFREE], mybir.dt.float16)
    ps = nc.alloc_psum_tensor("ps", [128, NFREE], mybir.dt.float32)

    sem_sg = [nc.alloc_semaphore(f"sem_sg{g}") for g in range(len(SYNC_GROUPS))]
    sem_fg = [nc.alloc_semaphore(f"sem_fg{g}") for g in range(len(F16_GROUPS))]
    sem_w = nc.alloc_semaphore("sem_w")
    sem_s = nc.alloc_semaphore("sem_s")
    sem_cv = {e: nc.alloc_semaphore(f"sem_cv_{e}") for e in CONV_ENGINES}
    pe_done = nc.alloc_semaphore("pe_done")
    ve_done = nc.alloc_semaphore("ve_done")
    odma = nc.alloc_semaphore("odma")

    def conv_ops(eng, ename):
        lastg = None
        for chunks, gi in runs_of[ename]:
            if gi != lastg:
                eng.wait_ge(sem_sg[gi], 16)
                lastg = gi
            lo, hi = chunks[0] * CW, (chunks[-1] + 1) * CW
            slo = lo - NF16 * CW
            shi = hi - NF16 * CW
            if ename == "scalar":
                op = eng.copy(out=y_sb[:, lo:hi], in_=y8_sb[:, slo:shi])
            else:
                op = eng.tensor_copy(out=y_sb[:, lo:hi], in_=y8_sb[:, slo:shi])
            op.then_inc(sem_cv[ename], 1)

    with nc.Block() as block:

        @block.sync
        def _(sync):
            for g, chunks in enumerate(SYNC_GROUPS):
                lo, hi = chunks[0] * CW, (chunks[-1] + 1) * CW
                slo = lo - NF16 * CW
                shi = hi - NF16 * CW
                sync.dma_start(
                    out=y8_sb[:, slo:shi], in_=y8[:, slo:shi]
                ).then_inc(sem_sg[g], 16)
            # dequant tile: only needed by the final tensor_mul, so it
            # queues behind the whole y stream without hurting anything
            sync.dma_start(out=s_sb[:, :], in_=s[:, :]).then_inc(sem_s, 16)
            sync.wait_ge(ve_done, 1)
            sync.dma_start(
                out=ut[:, :NFREE // 2], in_=u_sb[:, :NFREE // 2]
            ).then_inc(odma, 16)
            sync.wait_ge(odma, 32)

        @block.scalar
        def _(scalar):
            # W first (tensor engine blocks on it), then the fp16
            # chunks on this second HWDGE ring, then casts.
            scalar.dma_start(out=w_sb[:, :], in_=w[:, :]).then_inc(sem_w, 16)
            for g, chunks in enumerate(F16_GROUPS):
                lo, hi = chunks[0] * CW, (chunks[-1] + 1) * CW
                scalar.dma_start(
                    out=y_sb[:, lo:hi], in_=yf[:, lo:hi]
                ).then_inc(sem_fg[g], 16)
            conv_ops(scalar, "scalar")
            scalar.wait_ge(ve_done, 2)
            scalar.dma_start(
                out=ut[:, NFREE // 2:], in_=u_sb[:, NFREE // 2:]
            ).then_inc(odma, 16)
            scalar.wait_ge(odma, 32)

        @block.tensor
        def _(tensor):
            tensor.wait_ge(sem_w, 16)

            def wait_chunk(ci):
                if ci < NF16:
                    tensor.wait_ge(sem_fg[ci // 2], 16)
                else:
                    e, n = chunk_wait[ci]
                    tensor.wait_ge(sem_cv[e], n)

            for ci in range(NKC - 1):
                wait_chunk(ci)
                for bc in range(NB):
                    tensor.matmul(
                        ps[32 * bc:32 * bc + MC, :],
                        w_sb[:, ci * MC:(ci + 1) * MC],
                        y_sb[:, ci * CW + bc * NFREE:ci * CW + (bc + 1) * NFREE],
                        start=(ci == 0),
                        stop=False,
                        tile_position=(0, 32 * bc),
                    )
            # Last chunk in two N=256 halves so the dequant+store of the
            # first half overlaps the second half's matmuls.
            ci = NKC - 1
            wait_chunk(ci)
            for half in range(2):
                lo, hi = half * NFREE // 2, (half + 1) * NFREE // 2
                for bc in range(NB):
                    mm = tensor.matmul(
                        ps[32 * bc:32 * bc + MC, lo:hi],
                        w_sb[:, ci * MC:(ci + 1) * MC],
                        y_sb[:, ci * CW + bc * NFREE + lo:ci * CW + bc * NFREE + hi],
                        start=False,
                        stop=True,
                        tile_position=(0, 32 * bc),
                    )
                    mm.then_inc(pe_done, 1)

        @block.vector
        def _(vector):
            conv_ops(vector, "vector")
            vector.wait_ge(sem_s, 16)
            for half in range(2):
                lo, hi = half * NFREE // 2, (half + 1) * NFREE // 2
                vector.wait_ge(pe_done, NB * (half + 1))
                vector.tensor_mul(
                    out=u_sb[:, lo:hi], in0=ps[:, lo:hi], in1=s_sb[:, lo:hi]
                ).then_inc(ve_done, 1)

    _CACHE["nc"] = nc
    return nc


def _ensure_ntff_hook():
    """bass_utils hard-imports antenv.axon_hooks when BASS_TRACE is set;
    this container's trimmed antenv lacks it.  Register a working stub
    built from trn_agent_boot's ctypes NTFF driver (or a None hook,
    which bass_utils degrades gracefully on)."""
    import importlib.util
    import sys
    import types

    if "antenv.axon_hooks" in sys.modules:
        return
    try:
        if importlib.util.find_spec("antenv.axon_hooks") is not None:
            return
    except (ImportError, ValueError):
        pass
    try:
        from trn_agent_boot.trn_boot import _ntff_profile_via_ctypes

        hook = _ntff_profile_via_ctypes("/opt/axon/libaxon_pjrt.so")
    except Exception:
        hook = None
    mod = types.ModuleType("antenv.axon_hooks")
    mod.get_axon_ntff_profile_hook = lambda: hook
    sys.modules["antenv.axon_hooks"] = mod


def kernel(y_rev, M0, M_tilde, M_0l, M_big, sigma, lambda_e, phi, phi_tilde):
    _ensure_ntff_hook()
    from concourse.bass_utils import run_bass_kernel_spmd

    W = _build_w(M0, M_tilde, M_0l, M_big, sigma, lambda_e, phi, phi_tilde)
    # W_flat[k, c] with k = d*P + p, then swizzled so chunk ki sits at
    # columns [ki*MC, (ki+1)*MC) of a [128, NKC*MC] tile.
    Wf = W.transpose(0, 2, 1).reshape(K, MC)
    Wd = np.ascontiguousarray(
        Wf.reshape(NKC, 128, MC).transpose(1, 0, 2).reshape(128, NKC * MC)
    ).astype(np.float16)

    KF = NF16 * 128            # k-rows travelling as fp16
    in_maps = []
    for sh in range(NCORES):
        blk = y_rev[sh * BS:(sh + 1) * BS, :KD, :].reshape(BS, K)  # [b, k]
        srow = (np.abs(blk).max(axis=1) / 127.0).astype(np.float32)  # [BS]
        np.maximum(srow, 1e-30, out=srow)
        scaled = blk / srow[:, None]                               # |.| <= 127
        # chunks 0..NF16-1: pre-scaled fp16, partition-major
        yfp = np.ascontiguousarray(
            scaled[:, :KF].T.astype(np.float16)
            .reshape(NF16, 128, CW).transpose(1, 0, 2).reshape(128, NF16 * CW)
        )
        # chunks NF16..31: int8, partition-major
        q = np.rint(scaled[:, KF:])
        np.clip(q, -127, 127, out=q)
        q = q.astype(np.int8)
        ytp = np.ascontiguousarray(
            q.T.reshape(NKC - NF16, 128, CW).transpose(1, 0, 2)
            .reshape(128, (NKC - NF16) * CW)
        )
        stile = np.empty((128, NFREE), dtype=np.float32)
        for bc in range(NB):
            stile[32 * bc:32 * (bc + 1), :] = srow[None, bc * NFREE:(bc + 1) * NFREE]
        in_maps.append({"y8": ytp, "yf": yfp, "w": Wd, "s": stile})

    res = run_bass_kernel_spmd(_get_nc(), in_maps, list(range(NCORES)))
    _CACHE["last_result"] = res

    out = np.empty((B, MC), dtype=np.float32)
    for sh in range(NCORES):
        # ut[32*bc + c, j] = u^T[c, bc*512 + j]
        stripes = res.results[sh]["ut"].reshape(NB, 32, NFREE)[:, :MC, :]
        out[sh * BS:(sh + 1) * BS, :] = (
            stripes.transpose(0, 2, 1).reshape(BS, MC).astype(np.float32)
        )
    return out


# revision 25
# speedup vs baseline: 1.1177x; 1.1177x over previous
"""Trainium2 Bass kernel for nn_DSC_86071144612259.

The reference network collapses to a single linear contraction

    u[b, c] = sum_{d<128} sum_{p} W[d, p, c] * y_rev[b, d, p]

where W [128, P, MC] is assembled exactly (float64, on host) from the
small parameter tensors.  The 270 MB y_rev stream is the real work and
is purely DMA bound, so the kernel moves y as *int8* (per-batch-row
scale, absmax/127) -- half the HBM traffic of the fp16 baseline.  The
PE only eats float dtypes (the BIR verifier rejects integer matmuls),
so int8 y is upconverted to fp16 on-chip: the sync HWDGE ring streams
int8 (measured ~375 GB/s with 2 MB descriptors, the per-core HBM
share), and the casts are split between DVE tensor_copy (~1.22
us/chunk) and ACT activation-Copy (~2.0 us/chunk), which together
(~1.32 chunks/us) hide under the stream (~0.7 us/chunk).

The tensor engine chases per chunk with fp16 matmuls accumulating in
fp32 PSUM (4 batch blocks concurrently in disjoint 32-column PE
groups); the per-row dequant scale is applied by the final PSUM->SBUF
tensor_mul, fused with the output copy.  The only numeric loss is the
int8 rounding of y (measured absmax-rel ~9.2e-3 < the 2e-2 gate).

Sharding: pure data parallel over the batch axis across 8 cores (2048
rows each); W and the scale tile are replicated per-core inputs.
"""

import numpy as np

B = 16384      # batch
L = 129        # history length of y_rev
P = 32         # observation dim
MC = 16        # control dim (output)
H = 24         # spectral dim
M = 64         # filter length
NCORES = 8
BS = B // NCORES           # 2048 batch rows per core
KD = 128                   # delays with nonzero weight
K = KD * P                 # 4096 contraction length
NKC = K // 128             # 32 k-chunks of 128 partitions
CW = BS                    # SBUF columns per chunk (2048)
NFREE = 512                # matmul moving free dim (one fp32 PSUM bank)
NB = BS // NFREE           # 4 batch chunks per core

# All 32 chunks arrive int8 on the sync HWDGE ring and are cast to
# fp16 by DVE/ACT.  Groups stay <= 4 chunks (1 MB) so the converters
# never wait long on a batch; singles at head and tail.  GpSimd casts
# are NOT used: ~8 us/chunk AND they drag concurrent DVE casts down to
# the same pace (measured).  GpSimd DMAs (SWDGE Q0) are also out: any
# Q0 traffic collapses the concurrent HWDGE stream (134+161 vs 375
# solo).  A second HWDGE ring (ACT, Q10) is also out: it steals Q1
# bandwidth exactly during the early phase that feeds the converters.
SYNC_GROUPS = [[0], [1, 2], [3, 4], [5, 6, 7], [8, 9, 10, 11],
               [12, 13, 14, 15], [16, 17, 18, 19], [20, 21, 22, 23],
               [24, 25, 26, 27], [28, 29], [30], [31]]

# Measured batched cast rates (ns/chunk): DVE tensor_copy hits a 2x
# mode; ACT activation-Copy runs 1 elem/cycle.
CONV_RATE = {"vector": 1100.0, "scalar": 1830.0}
CONV_ENGINES = ("vector", "scalar")

# Measured Q1 arrival curve (cumulative MB by us) from the HW trace:
# slow queue ramp, then ~420 B/ns steady.
_ARRIVAL = [(10.0, 0.0), (12.0, 0.45), (14.0, 1.2), (16.0, 2.1),
            (18.0, 2.95), (20.0, 3.85), (22.0, 4.75), (24.0, 5.6),
            (26.0, 6.45), (28.0, 7.25), (30.0, 8.05), (31.6, 8.45)]

_CACHE = {}


def _land_time(cum_mb):
    for (t0, b0), (t1, b1) in zip(_ARRIVAL, _ARRIVAL[1:]):
        if cum_mb <= b1:
            return t0 + (t1 - t0) * (cum_mb - b0) / (b1 - b0)
    return _ARRIVAL[-1][0]


def _conv_runs():
    """Greedy DVE/ACT assignment of chunk casts against the measured
    arrival curve, coalescing adjacent same-engine chunks of a group
    into one batched op.  Chunk 31 is forced onto DVE (faster)."""
    grp = {}
    land = {}
    cum = 0.0
    for gi, chunks in enumerate(SYNC_GROUPS):
        cum += len(chunks) * 0.2621
        for ci in chunks:
            grp[ci] = gi
            land[ci] = _land_time(cum) * 1000.0
    free = {e: 10000.0 for e in CONV_ENGINES}
    assign = {}
    for ci in range(NKC):
        if ci == NKC - 1:
            e = "vector"
        else:
            e = min(CONV_ENGINES,
                    key=lambda e: max(free[e], land[ci]) + CONV_RATE[e])
        assign[ci] = e
        free[e] = max(free[e], land[ci]) + CONV_RATE[e]
    runs = []
    for ci in range(NKC):
        if runs and runs[-1][0] == assign[ci] and runs[-1][2] == grp[ci] \
                and runs[-1][1][-1] == ci - 1:
            runs[-1][1].append(ci)
        else:
            runs.append((assign[ci], [ci], grp[ci]))
    return runs


def _build_w(M0, M_tilde, M_0l, M_big, sigma, lambda_e, phi, phi_tilde):
    """Collapse the parameter tensors into W [KD, MC, P] (float64).

    Mirrors reference.py exactly:
      term1: delay 0,      M0
      term2: delays 1..64, sum_i lambda_i^0.25 phi_tilde[j-1,i] M_tilde[i]
      term3: delays 0..63, sum_l sigma_l^0.25  phi[k,l]         M_0l[l]
      term4: delays 1..127 via conv(phi_tilde[:,i], phi[:,l]) and M_big
    """
    f8 = np.float64
    M0 = M0.astype(f8)
    M_tilde = M_tilde.astype(f8)
    M_0l = M_0l.astype(f8)
    M_big = M_big.astype(f8)
    sigma = sigma.astype(f8)
    lambda_e = lambda_e.astype(f8)
    phi = phi.astype(f8)
    phi_tilde = phi_tilde.astype(f8)

    W = np.zeros((KD, MC, P), dtype=f8)
    W[0] += M0
    pt = phi_tilde * (lambda_e ** 0.25)[None, :]
    W[1:M + 1] += np.einsum("ji,icp->jcp", pt, M_tilde)
    ps = phi * (sigma ** 0.25)[None, :]
    W[0:M] += np.einsum("kl,lcp->kcp", ps, M_0l)
    W4 = np.empty((H, H, 2 * M - 1), dtype=f8)
    for i in range(H):
        for l in range(H):
            W4[i, l] = np.convolve(phi_tilde[:, i], phi[:, l])
    scale = (lambda_e[:, None] * sigma[None, :]) ** 0.25
    W[1:2 * M] += np.einsum("ild,ilcp->dcp", W4 * scale[:, :, None], M_big)
    return W


def _get_nc():
    """Build the per-core Bass program (cached)."""
    if "nc" in _CACHE:
        return _CACHE["nc"]
    import concourse.bass as bass
    import concourse.mybir as mybir

    # per-chunk: (engine, run-ordinal on that engine) for matmul waits
    chunk_wait = {}
    runs_of = {e: [] for e in CONV_ENGINES}
    for ename, chunks, gi in _conv_runs():
        runs_of[ename].append((chunks, gi))
        for ci in chunks:
            chunk_wait[ci] = (ename, len(runs_of[ename]))
    assert sorted(chunk_wait) == list(range(NKC))

    nc = bass.Bass("TRN2", target_bir_lowering=False, enable_partition_id=False)
    y8 = nc.dram_tensor("y8", [128, NKC * CW], mybir.dt.int8, kind="ExternalInput")
    w = nc.dram_tensor("w", [128, NKC * MC], mybir.dt.float16, kind="ExternalInput")
    s = nc.dram_tensor("s", [128, NFREE], mybir.dt.float32, kind="ExternalInput")
    ut = nc.dram_tensor("ut", [128, NFREE], mybir.dt.float16, kind="ExternalOutput")

    y8_sb = nc.alloc_sbuf_tensor("y8_sb", [128, NKC * CW], mybir.dt.int8)
    y_sb = nc.alloc_sbuf_tensor("y_sb", [128, NKC * CW], mybir.dt.float16)
    # W pre-swizzled on host: w_sb[p, ki*MC + c] = W_flat[ki*128 + p, c]
    w_sb = nc.alloc_sbuf_tensor("w_sb", [128, NKC * MC], mybir.dt.float16)
    # Dequant tile: s_sb[32*bc + c, j] = s_row[bc*512 + j]
    s_sb = nc.alloc_sbuf_tensor("s_sb", [128, NFREE], mybir.dt.float32)
    # Output striped across partitions: row 32*bc + c holds u^T[c, bc*512+j]
    u_sb = nc.alloc_sbuf_tensor("u_sb", [128, NFREE], mybir.dt.float16)
    # scratch for the ACT activation-table preload dummy
    warm_sb = nc.alloc_sbuf_tensor("warm_sb", [128, 4], mybir.dt.float16)
    ps = nc.alloc_psum_tensor("ps", [128, NFREE], mybir.dt.float32)

    sem_sg = [nc.alloc_semaphore(f"sem_sg{g}") for g in range(len(SYNC_GROUPS))]
    sem_w = nc.alloc_semaphore("sem_w")
    sem_s = nc.alloc_semaphore("sem_s")
    sem_cv = {e: nc.alloc_semaphore(f"sem_cv_{e}") for e in CONV_ENGINES}
    pe_done = nc.alloc_semaphore("pe_done")
    ve_done = nc.alloc_semaphore("ve_done")
    odma = nc.alloc_semaphore("odma")

    def conv_ops(eng, ename):
        lastg = None
        for chunks, gi in runs_of[ename]:
            if gi != lastg:
                eng.wait_ge(sem_sg[gi], 16)
                lastg = gi
            lo, hi = chunks[0] * CW, (chunks[-1] + 1) * CW
            if ename == "scalar":
                op = eng.copy(out=y_sb[:, lo:hi], in_=y8_sb[:, lo:hi])
            else:
                op = eng.tensor_copy(out=y_sb[:, lo:hi], in_=y8_sb[:, lo:hi])
            op.then_inc(sem_cv[ename], 1)

    with nc.Block() as block:

        @block.sync
        def _(sync):
            for g, chunks in enumerate(SYNC_GROUPS):
                lo, hi = chunks[0] * CW, (chunks[-1] + 1) * CW
                sync.dma_start(
                    out=y8_sb[:, lo:hi], in_=y8[:, lo:hi]
                ).then_inc(sem_sg[g], 16)
            sync.wait_ge(ve_done, 1)
            sync.dma_start(
                out=ut[:, :NFREE // 2], in_=u_sb[:, :NFREE // 2]
            ).then_inc(odma, 16)
            sync.wait_ge(odma, 32)

        @block.gpsimd
        def _(gpsimd):
            # the dequant tile is only needed by the final tensor_mul;
            # park its DMA on the otherwise idle gpsimd SWDGE queue
            gpsimd.dma_start(out=s_sb[:, :], in_=s[:, :]).then_inc(sem_s, 16)

        @block.scalar
        def _(scalar):
            # W first (tensor engine blocks on it); then a dummy Copy
            # to pull the ~1.3 us activation-table load out of the
            # first cast's critical path (reads garbage, result unused).
            scalar.dma_start(out=w_sb[:, :], in_=w[:, :]).then_inc(sem_w, 16)
            scalar.copy(out=warm_sb[:, :], in_=y8_sb[:, 0:4])
            conv_ops(scalar, "scalar")
            scalar.wait_ge(ve_done, 2)
            scalar.dma_start(
                out=ut[:, NFREE // 2:], in_=u_sb[:, NFREE // 2:]
            ).then_inc(odma, 16)
            scalar.wait_ge(odma, 32)

        @block.tensor
        def _(tensor):
            tensor.wait_ge(sem_w, 16)

            def wait_chunk(ci):
                e, n = chunk_wait[ci]
                tensor.wait_ge(sem_cv[e], n)

            for ci in range(NKC - 1):
                wait_chunk(ci)
                for bc in range(NB):
                    tensor.matmul(
                        ps[32 * bc:32 * bc + MC, :],
                        w_sb[:, ci * MC:(ci + 1) * MC],
                        y_sb[:, ci * CW + bc * NFREE:ci * CW + (bc + 1) * NFREE],
                        start=(ci == 0),
                        stop=False,
                        tile_position=(0, 32 * bc),
                    )
            # Last chunk in two N=256 halves so the dequant+store of the
            # first half overlaps the second half's matmuls.
            ci = NKC - 1
            wait_chunk(ci)
            for half in range(2):
                lo, hi = half * NFREE // 2, (half + 1) * NFREE // 2
                for bc in range(NB):
                    mm = tensor.matmul(
                        ps[32 * bc:32 * bc + MC, lo:hi],
                        w_sb[:, ci * MC:(ci + 1) * MC],
                        y_sb[:, ci * CW + bc * NFREE + lo:ci * CW + bc * NFREE + hi],
                        start=False,
                        stop=True,
                        tile_position=(0, 32 * bc),
                    )
                    mm.then_inc(pe_done, 1)

        @block.vector
        def _(vector):
            conv_ops(vector, "vector")
            vector.wait_ge(sem_s, 16)
            for half in range(2):
                lo, hi = half * NFREE // 2, (half + 1) * NFREE // 2
                vector.wait_ge(pe_done, NB * (half + 1))
                vector.tensor_mul(
                    out=u_sb[:, lo:hi], in0=ps[:, lo:hi], in1=s_sb[:, lo:hi]
                ).then_inc(ve_done, 1)

    _CACHE["nc"] = nc
    return nc


def _ensure_ntff_hook():
    """bass_utils hard-imports antenv.axon_hooks when BASS_TRACE is set;
    this container's trimmed antenv lacks it.  Register a working stub
    built from trn_agent_boot's ctypes NTFF driver (or a None hook,
    which bass_utils degrades gracefully on)."""
    import importlib.util
    import sys
    import types

    if "antenv.axon_hooks" in sys.modules:
        return
    try:
        if importlib.util.find_spec("antenv.axon_hooks") is not None:
            return
    except (ImportError, ValueError):
        pass
    try:
        from trn_agent_boot.trn_boot import _ntff_profile_via_ctypes

        hook = _ntff_profile_via_ctypes("/opt/axon/libaxon_pjrt.so")
    except Exception:
        hook = None
    mod = types.ModuleType("antenv.axon_hooks")
    mod.get_axon_ntff_profile_hook = lambda: hook
    sys.modules["antenv.axon_hooks"] = mod


def kernel(y_rev, M0, M_tilde, M_0l, M_big, sigma, lambda_e, phi, phi_tilde):
    _ensure_ntff_hook()
    from concourse.bass_utils import run_bass_kernel_spmd

    W = _build_w(M0, M_tilde, M_0l, M_big, sigma, lambda_e, phi, phi_tilde)
    # W_flat[k, c] with k = d*P + p, then swizzled so chunk ki sits at
    # columns [ki*MC, (ki+1)*MC) of a [128, NKC*MC] tile.
    Wf = W.transpose(0, 2, 1).reshape(K, MC)
    Wd = np.ascontiguousarray(
        Wf.reshape(NKC, 128, MC).transpose(1, 0, 2).reshape(128, NKC * MC)
    ).astype(np.float16)

    in_maps = []
    for sh in range(NCORES):
        blk = y_rev[sh * BS:(sh + 1) * BS, :KD, :].reshape(BS, K)  # [b, k]
        srow = (np.abs(blk).max(axis=1) / 127.0).astype(np.float32)  # [BS]
        np.maximum(srow, 1e-30, out=srow)
        q = np.rint(blk / srow[:, None])
        np.clip(q, -127, 127, out=q)
        q = q.astype(np.int8)
        # partition-major DRAM layout: y8[p, ki*CW + j] = q[j, ki*128 + p]
        ytp = np.ascontiguousarray(
            q.T.reshape(NKC, 128, CW).transpose(1, 0, 2).reshape(128, NKC * CW)
        )
        stile = np.empty((128, NFREE), dtype=np.float32)
        for bc in range(NB):
            stile[32 * bc:32 * (bc + 1), :] = srow[None, bc * NFREE:(bc + 1) * NFREE]
        in_maps.append({"y8": ytp, "w": Wd, "s": stile})

    res = run_bass_kernel_spmd(_get_nc(), in_maps, list(range(NCORES)))
    _CACHE["last_result"] = res

    out = np.empty((B, MC), dtype=np.float32)
    for sh in range(NCORES):
        # ut[32*bc + c, j] = u^T[c, bc*512 + j]
        stripes = res.results[sh]["ut"].reshape(NB, 32, NFREE)[:, :MC, :]
        out[sh * BS:(sh + 1) * BS, :] = (
            stripes.transpose(0, 2, 1).reshape(BS, MC).astype(np.float32)
        )
    return out
